# revision 9
# baseline (speedup 1.0000x reference)
"""Additive (Bahdanau) attention on 8 TRN2 NeuronCores.

Reference computation (per batch element b, one NeuronCore each):
    q  = queries @ W_q.T                      # (Q, H)
    k  = keys @ W_k.T                         # (K, H)
    s[q,k] = sum_h w_v[h] * tanh(q[q,h] + k[k,h])
    s[q,k] += mask (0 valid / -big invalid)
    attn = softmax_k(s)
    out  = attn @ values                      # (Q, Dv)

Shapes: B=8, Q=128, K=1024, D=256, H=256 (hardcoded; data-parallel over B).

Device strategy per core:
  * host pre-transposes/casts small operands to fp16 (qT, kT, W_q^T, W_k^T)
  * TensorE computes qf^T (h,q) into SBUF f32 and kf^T (h,k) into PSUM f32
  * main loop over (q, h_tile): ScalarE computes
        T = tanh(kfT_psum + bias=qfT[:, q])     (one (128,1024) activation)
    and TensorE reduces over h with w_v via a sliding-window one-hot-column
    stationary, accumulating scores straight into a (q,k) PSUM tile.
  * mask folded in as a rank-1 matmul accumulate; softmax via reduce_max /
    activation(Exp, bias=-max, accum_out=rowsum); attn @ V via PE transposes.

ScalarE (tanh of Q*K*H = 33.5M elements) is the roofline engine.
"""

import numpy as np

B, Q, K, D, H = 8, 128, 1024, 256, 256
NEG_BIG = -60000.0  # fp16-representable "minus infinity" for masking

_CACHE = {}


def _build_bass():
    import concourse.bass as bass
    import concourse.tile as tile
    from concourse import mybir
    from concourse.masks import make_identity
    from contextlib import ExitStack

    F32 = mybir.dt.float32
    F16 = mybir.dt.float16
    AF = mybir.ActivationFunctionType

    nc = bass.Bass(use_seq_codegen=True)

    qT_ext = nc.declare_dram_parameter("qT", [D, Q], F16, isOutput=False)
    kT_ext = nc.declare_dram_parameter("kT", [D, K], F16, isOutput=False)
    vals_ext = nc.declare_dram_parameter("vals", [K, D], F16, isOutput=False)
    wqT_ext = nc.declare_dram_parameter("wqT", [D, H], F16, isOutput=False)
    wkT_ext = nc.declare_dram_parameter("wkT", [D, H], F16, isOutput=False)
    wv_ext = nc.declare_dram_parameter("wv_win", [2, 128, 255], F16, isOutput=False)
    mask_ext = nc.declare_dram_parameter("mask", [1, K], F16, isOutput=False)
    out_ext = nc.declare_dram_parameter("out", [Q, D], F32, isOutput=True)

    with tile.TileContext(nc) as tc, ExitStack() as ctx:
        persist = ctx.enter_context(tc.tile_pool(name="persist", bufs=1))
        scores_ps = ctx.enter_context(tc.tile_pool(name="scores_ps", bufs=1, space="PSUM"))
        t_pool = ctx.enter_context(tc.tile_pool(name="t_pool", bufs=4))

        # ---- persistent SBUF tiles ----
        qT_sb = persist.tile([128, 2, Q], F16)      # [d_in_tile, d_tile, q]
        kT_sb = persist.tile([128, 2, K], F16)
        wqT_sb = persist.tile([128, 2, H], F16)
        wkT_sb = persist.tile([128, 2, H], F16)
        wv_sb = persist.tile([128, 2, 255], F16)    # sliding-window w_v columns
        val_sb = persist.tile([128, 8, D], F16)     # [k_in_tile, k_tile, v]
        mask_sb = persist.tile([1, K], F16)
        ones_sb = persist.tile([1, 128], F16)
        ident = persist.tile([128, 128], F16)
        qfT_sb = persist.tile([128, 2, Q], F32)     # [h_in_tile, h_tile, q]
        E_sb = persist.tile([128, K], F16)          # exp(scores - max), (q, k)
        ET_sb = persist.tile([128, 8, 128], F16)    # transposed E, [k_in_tile, k_tile, q]
        out_sb = persist.tile([Q, D], F32)
        rowmax = persist.tile([128, 1], F32)
        negmax = persist.tile([128, 1], F32)
        rowsum = persist.tile([128, 1], F32)
        rinv = persist.tile([128, 1], F32)

        # ---- DMA inputs ----
        nc.sync.dma_start(out=qT_sb, in_=qT_ext.rearrange("(t p) q -> p t q", p=128))
        nc.sync.dma_start(out=kT_sb, in_=kT_ext.rearrange("(t p) k -> p t k", p=128))
        nc.sync.dma_start(out=wqT_sb, in_=wqT_ext.rearrange("(t p) h -> p t h", p=128))
        nc.sync.dma_start(out=wkT_sb, in_=wkT_ext.rearrange("(t p) h -> p t h", p=128))
        nc.sync.dma_start(out=wv_sb, in_=wv_ext.rearrange("t p c -> p t c"))
        nc.sync.dma_start(out=val_sb, in_=vals_ext.rearrange("(t p) v -> p t v", p=128))
        nc.sync.dma_start(out=mask_sb, in_=mask_ext[:, :])
        nc.vector.memset(ones_sb, 1.0)
        make_identity(nc, ident)

        # ---- scores PSUM (q, k) over 2 banks; mask as rank-1 accumulate ----
        scores = scores_ps.tile([128, K], F32)
        for c in range(2):
            csl = slice(c * 512, (c + 1) * 512)
            nc.tensor.matmul(scores[:, csl], ones_sb, mask_sb[:, csl],
                             start=True, stop=False)

        with ExitStack() as main_ctx:
            setup_ps = main_ctx.enter_context(
                tc.tile_pool(name="setup_ps", bufs=1, space="PSUM"))
            kf_ps = main_ctx.enter_context(
                tc.tile_pool(name="kf_ps", bufs=1, space="PSUM"))

            # ---- projections: qfT[h, q] (SBUF f32) and kfT[h, k] (PSUM f32) ----
            ps_q = setup_ps.tile([128, 2 * Q], F32)
            for ht in range(2):
                hsl = slice(ht * 128, (ht + 1) * 128)
                qsl = slice(ht * Q, (ht + 1) * Q)
                nc.tensor.matmul(ps_q[:, qsl], wqT_sb[:, 0, hsl], qT_sb[:, 0, :],
                                 start=True, stop=False)
                nc.tensor.matmul(ps_q[:, qsl], wqT_sb[:, 1, hsl], qT_sb[:, 1, :],
                                 start=False, stop=True)
            nc.vector.tensor_copy(qfT_sb, ps_q.rearrange("p (t q) -> p t q", t=2))

            kf0 = kf_ps.tile([128, K], F32, tag="kf0")
            kf1 = kf_ps.tile([128, K], F32, tag="kf1")
            kf = [kf0, kf1]
            for ht in range(2):
                hsl = slice(ht * 128, (ht + 1) * 128)
                for c in range(2):
                    csl = slice(c * 512, (c + 1) * 512)
                    nc.tensor.matmul(kf[ht][:, csl], wkT_sb[:, 0, hsl],
                                     kT_sb[:, 0, csl], start=True, stop=False)
                    nc.tensor.matmul(kf[ht][:, csl], wkT_sb[:, 1, hsl],
                                     kT_sb[:, 1, csl], start=False, stop=True)

            # ---- main loop: biased tanh + weighted h-reduction ----
            for q in range(Q):
                for ht in range(2):
                    tt = t_pool.tile([128, K], F16, tag="tt")
                    nc.scalar.activation(tt, kf[ht], AF.Tanh,
                                         bias=qfT_sb[:, ht, q:q + 1], scale=1.0)
                    last = (q == Q - 1) and (ht == 1)
                    for c in range(2):
                        csl = slice(c * 512, (c + 1) * 512)
                        nc.tensor.matmul(scores[:, csl],
                                         wv_sb[:, ht, 127 - q:255 - q],
                                         tt[:, csl], start=False, stop=last)

        # ---- masked softmax ----
        nc.vector.tensor_reduce(rowmax, scores, axis=mybir.AxisListType.X,
                                op=mybir.AluOpType.max)
        nc.vector.tensor_scalar_mul(negmax, rowmax, -1.0)
        nc.scalar.activation(E_sb, scores, AF.Exp, bias=negmax, scale=1.0,
                             accum_out=rowsum)
        nc.vector.reciprocal(rinv, rowsum)

        # ---- attn @ values: transpose E, then accumulate over k tiles ----
        with ExitStack() as tail_ctx:
            tp_ps = tail_ctx.enter_context(
                tc.tile_pool(name="tp_ps", bufs=2, space="PSUM"))
            av_ps = tail_ctx.enter_context(
                tc.tile_pool(name="av_ps", bufs=1, space="PSUM"))
            for kt in range(8):
                tp = tp_ps.tile([128, 128], F16, tag="tp")
                nc.tensor.transpose(tp, E_sb[:, kt * 128:(kt + 1) * 128], ident)
                nc.vector.tensor_copy(ET_sb[:, kt, :], tp)
            ps_av = av_ps.tile([Q, D], F32)
            for kt in range(8):
                nc.tensor.matmul(ps_av, ET_sb[:, kt, :], val_sb[:, kt, :],
                                 start=(kt == 0), stop=(kt == 7))
            nc.vector.tensor_scalar_mul(out_sb, ps_av, rinv)
        nc.sync.dma_start(out=out_ext[:, :], in_=out_sb)

    _patch_multiwait(nc)
    return nc


def _patch_multiwait(nc):
    """walrus codegen on this toolchain accepts at most ONE sync wait per
    compute-engine instruction ("Too many sync wait commands").  Tile emits
    up to 3.  Fix the serialized BIR:

    * Activation-engine instructions waiting on the Activation semaphore:
      that wait is transitively implied (the tile slot's PE readers already
      gated on it), drop it.
    * Any remaining instruction with >1 waits: hoist all but the last onto
      single-wait EventSemaphore carrier instructions inserted just before
      it on the same engine queue (queue is in-order, so semantics match).
    """
    import json

    d = json.loads(nc.to_json_bytes())
    k = [0]
    keep_multi = ("EventSemaphore",)
    for fn in d["functions"]:
        for blk in fn["blocks"]:
            out = []
            for inst in blk["instructions"]:
                si = inst.get("sync_info") or {}
                ow = si.get("on_wait") or []
                op = inst.get("opcode")
                eng = inst.get("engine")
                if len(ow) > 1 and op not in keep_multi:
                    if eng == "Activation":
                        ow2 = [w for w in ow
                               if not str(w.get("ant_name", "")).startswith("Activation")]
                        if ow2:
                            ow = ow2
                    while len(ow) > 1:
                        w = ow.pop(0)
                        k[0] += 1
                        out.append({
                            "debug": inst.get("debug", 0), "engine": eng,
                            "ins": [], "name": f"WSplit-{k[0]}",
                            "opcode": "EventSemaphore", "outs": [],
                            "sync_info": {"on_update": [], "on_wait": [w]},
                        })
                    si["on_wait"] = ow
                out.append(inst)
            blk["instructions"] = out
    patched = json.dumps(d).encode()
    nc.to_json_bytes = lambda: patched


def _get_nc():
    if "nc" not in _CACHE:
        _CACHE["nc"] = _build_bass()
    return _CACHE["nc"]


def _host_prep(queries, keys, values, W_q, W_k, w_v, valid_lens):
    """Build the 8 per-core input maps."""
    queries = np.asarray(queries, dtype=np.float32)
    keys = np.asarray(keys, dtype=np.float32)
    values = np.asarray(values, dtype=np.float32)
    W_q = np.asarray(W_q, dtype=np.float32)
    W_k = np.asarray(W_k, dtype=np.float32)
    w_v = np.asarray(w_v, dtype=np.float32)
    valid = np.asarray(valid_lens).astype(np.int64)

    wqT = np.ascontiguousarray(W_q.T.astype(np.float16))     # (d, h)
    wkT = np.ascontiguousarray(W_k.T.astype(np.float16))
    wv_win = np.zeros((2, 128, 255), dtype=np.float16)
    wv_win[0, :, 127] = w_v[:128].astype(np.float16)
    wv_win[1, :, 127] = w_v[128:].astype(np.float16)

    kidx = np.arange(K)
    in_maps = []
    for b in range(B):
        mask = np.where(kidx < valid[b], np.float16(0.0), np.float16(NEG_BIG))
        in_maps.append({
            "qT": np.ascontiguousarray(queries[b].T.astype(np.float16)),
            "kT": np.ascontiguousarray(keys[b].T.astype(np.float16)),
            "vals": np.ascontiguousarray(values[b].astype(np.float16)),
            "wqT": wqT,
            "wkT": wkT,
            "wv_win": wv_win,
            "mask": np.ascontiguousarray(mask.reshape(1, K)),
        })
    return in_maps, valid, values


def _run(inputs, trace=False, **kw):
    from concourse.bass_utils import run_bass_kernel_spmd

    nc = _get_nc()
    in_maps, valid, values = _host_prep(**inputs)
    res = run_bass_kernel_spmd(nc, in_maps, list(range(B)), trace=trace, **kw)
    out = np.stack([np.asarray(res.results[i]["out"], dtype=np.float32)
                    for i in range(B)])
    # valid_len == 0 -> reference softmax over an all -1e9 row is uniform 1/K
    for b in range(B):
        if valid[b] == 0:
            out[b] = np.broadcast_to(values[b].mean(axis=0), (Q, D))
    return out, res


def kernel(**inputs):
    out, _ = _run(inputs, trace=False)
    return out


# revision 10
# speedup vs baseline: 1.0910x; 1.0910x over previous
"""Additive (Bahdanau) attention on 8 TRN2 NeuronCores.

Reference computation (per batch element b, one NeuronCore each):
    q  = queries @ W_q.T                      # (Q, H)
    k  = keys @ W_k.T                         # (K, H)
    s[q,k] = sum_h w_v[h] * tanh(q[q,h] + k[k,h])
    s[q,k] += mask (0 valid / -big invalid)
    attn = softmax_k(s)
    out  = attn @ values                      # (Q, Dv)

Shapes: B=8, Q=128, K=1024, D=256, H=256 (hardcoded; data-parallel over B).

Device strategy per core:
  * host pre-transposes/casts small operands to fp16 (qT, kT, W_q^T, W_k^T)
  * TensorE computes qf^T (h,q) and kf^T (h,k) projections
  * main loop over groups of G queries x 2 h-tiles:
      - VectorE  : arg[h, j*1024+k] = kfT[h,k] + qfT[h,q_j]  (tensor_scalar 4x)
      - ScalarE  : one (128, G*1024) tanh activation         (roofline engine)
      - TensorE  : h-reduction with w_v via a sliding-window one-hot-column
                   stationary, accumulating scores straight into (q,k) PSUM
  * mask folded in as a rank-1 matmul accumulate; softmax via reduce_max /
    activation(Exp, bias=-max, accum_out=rowsum); attn @ V via PE transposes.

ScalarE (tanh of Q*K*H = 33.5M elements) is the roofline engine: ~219us of
pure streaming at 1.2GHz/128 lanes; batching G queries per activation
amortizes the ~350-cycle per-instruction overhead.
"""

import numpy as np

B, Q, K, D, H = 8, 128, 1024, 256, 256
GQ = 8          # queries per activation instruction
NEG_BIG = -60000.0  # fp16-representable "minus infinity" for masking

_CACHE = {}


def _build_bass():
    import concourse.bass as bass
    import concourse.tile as tile
    from concourse import mybir
    from concourse.masks import make_identity
    from contextlib import ExitStack

    F32 = mybir.dt.float32
    F16 = mybir.dt.float16
    AF = mybir.ActivationFunctionType

    nc = bass.Bass()

    qT_ext = nc.declare_dram_parameter("qT", [D, Q], F16, isOutput=False)
    kT_ext = nc.declare_dram_parameter("kT", [D, K], F16, isOutput=False)
    vals_ext = nc.declare_dram_parameter("vals", [K, D], F16, isOutput=False)
    wqT_ext = nc.declare_dram_parameter("wqT", [D, H], F16, isOutput=False)
    wkT_ext = nc.declare_dram_parameter("wkT", [D, H], F16, isOutput=False)
    wv_ext = nc.declare_dram_parameter("wv_win", [2, 128, 255], F16, isOutput=False)
    mask_ext = nc.declare_dram_parameter("mask", [1, K], F16, isOutput=False)
    out_ext = nc.declare_dram_parameter("out", [Q, D], F32, isOutput=True)

    with tile.TileContext(nc) as tc, ExitStack() as ctx:
        persist = ctx.enter_context(tc.tile_pool(name="persist", bufs=1))
        scores_ps = ctx.enter_context(tc.tile_pool(name="scores_ps", bufs=1, space="PSUM"))
        arg_pool = ctx.enter_context(tc.tile_pool(name="arg_pool", bufs=3))
        t_pool = ctx.enter_context(tc.tile_pool(name="t_pool", bufs=3))

        # ---- persistent SBUF tiles ----
        qT_sb = persist.tile([128, 2, Q], F16)      # [d_in_tile, d_tile, q]
        kT_sb = persist.tile([128, 2, K], F16)
        wqT_sb = persist.tile([128, 2, H], F16)
        wkT_sb = persist.tile([128, 2, H], F16)
        wv_sb = persist.tile([128, 2, 255], F16)    # sliding-window w_v columns
        val_sb = persist.tile([128, 8, D], F16)     # [k_in_tile, k_tile, v]
        mask_sb = persist.tile([1, K], F16)
        ones_sb = persist.tile([1, 128], F16)
        ident = persist.tile([128, 128], F16)
        qfT_sb = persist.tile([128, 2, Q], F32)     # [h_in_tile, h_tile, q]
        kf_sb = persist.tile([128, 2, K], F16)
        E_sb = persist.tile([128, K], F16)          # exp(scores - max), (q, k)
        ET_sb = persist.tile([128, 8, 128], F16)    # transposed E, [k_in_tile, k_tile, q]
        out_sb = persist.tile([Q, D], F32)
        rowmax = persist.tile([128, 1], F32)
        negmax = persist.tile([128, 1], F32)
        rowsum = persist.tile([128, 1], F32)
        rinv = persist.tile([128, 1], F32)

        # ---- DMA inputs (projection-critical tensors first) ----
        nc.sync.dma_start(out=qT_sb, in_=qT_ext.rearrange("(t p) q -> p t q", p=128))
        nc.sync.dma_start(out=wqT_sb, in_=wqT_ext.rearrange("(t p) h -> p t h", p=128))
        nc.sync.dma_start(out=wkT_sb, in_=wkT_ext.rearrange("(t p) h -> p t h", p=128))
        for t in range(2):
            nc.sync.dma_start(out=kT_sb[:, t, :], in_=kT_ext[t * 128:(t + 1) * 128, :])
        nc.sync.dma_start(out=wv_sb, in_=wv_ext.rearrange("t p c -> p t c"))
        nc.sync.dma_start(out=mask_sb, in_=mask_ext[:, :])
        nc.sync.dma_start(out=val_sb, in_=vals_ext.rearrange("(t p) v -> p t v", p=128))
        nc.vector.memset(ones_sb, 1.0)
        make_identity(nc, ident)

        # ---- scores PSUM (q, k) over 2 banks; mask as rank-1 accumulate ----
        scores = scores_ps.tile([128, K], F32)
        for c in range(2):
            csl = slice(c * 512, (c + 1) * 512)
            nc.tensor.matmul(scores[:, csl], ones_sb, mask_sb[:, csl],
                             start=True, stop=False)

        with ExitStack() as setup_ctx:
            setup_ps = setup_ctx.enter_context(
                tc.tile_pool(name="setup_ps", bufs=1, space="PSUM"))
            kf_ps = setup_ctx.enter_context(
                tc.tile_pool(name="kf_ps", bufs=1, space="PSUM"))

            # ---- projections: qfT[h, q] and kfT[h, k] -> SBUF ----
            ps_q = setup_ps.tile([128, 2 * Q], F32)
            for ht in range(2):
                hsl = slice(ht * 128, (ht + 1) * 128)
                qsl = slice(ht * Q, (ht + 1) * Q)
                nc.tensor.matmul(ps_q[:, qsl], wqT_sb[:, 0, hsl], qT_sb[:, 0, :],
                                 start=True, stop=False)
                nc.tensor.matmul(ps_q[:, qsl], wqT_sb[:, 1, hsl], qT_sb[:, 1, :],
                                 start=False, stop=True)
            nc.vector.tensor_copy(qfT_sb, ps_q.rearrange("p (t q) -> p t q", t=2))

            kf0 = kf_ps.tile([128, K], F32, tag="kf0")
            kf1 = kf_ps.tile([128, K], F32, tag="kf1")
            kf = [kf0, kf1]
            for ht in range(2):
                hsl = slice(ht * 128, (ht + 1) * 128)
                for c in range(2):
                    csl = slice(c * 512, (c + 1) * 512)
                    nc.tensor.matmul(kf[ht][:, csl], wkT_sb[:, 0, hsl],
                                     kT_sb[:, 0, csl], start=True, stop=False)
                    nc.tensor.matmul(kf[ht][:, csl], wkT_sb[:, 1, hsl],
                                     kT_sb[:, 1, csl], start=False, stop=True)
                nc.vector.tensor_copy(kf_sb[:, ht, :], kf[ht])

        # ---- main loop: add + tanh (batched over GQ queries) + h-reduction ----
        n_groups = Q // GQ
        for g in range(n_groups):
            for ht in range(2):
                arg = arg_pool.tile([128, GQ * K], F16, tag="arg")
                for j in range(GQ):
                    q = g * GQ + j
                    nc.vector.tensor_scalar_add(
                        arg[:, j * K:(j + 1) * K], kf_sb[:, ht, :],
                        qfT_sb[:, ht, q:q + 1])
                tt = t_pool.tile([128, GQ * K], F16, tag="tt")
                nc.scalar.activation(tt, arg, AF.Tanh)
                for j in range(GQ):
                    q = g * GQ + j
                    last = (g == n_groups - 1) and (ht == 1) and (j == GQ - 1)
                    for c in range(2):
                        csl = slice(c * 512, (c + 1) * 512)
                        nc.tensor.matmul(
                            scores[:, csl],
                            wv_sb[:, ht, 127 - q:255 - q],
                            tt[:, j * K + c * 512: j * K + (c + 1) * 512],
                            start=False, stop=last)

        # ---- masked softmax ----
        nc.vector.tensor_reduce(rowmax, scores, axis=mybir.AxisListType.X,
                                op=mybir.AluOpType.max)
        nc.vector.tensor_scalar_mul(negmax, rowmax, -1.0)
        nc.scalar.activation(E_sb, scores, AF.Exp, bias=negmax, scale=1.0,
                             accum_out=rowsum)
        nc.vector.reciprocal(rinv, rowsum)

        # ---- attn @ values: transpose E, then accumulate over k tiles ----
        with ExitStack() as tail_ctx:
            tp_ps = tail_ctx.enter_context(
                tc.tile_pool(name="tp_ps", bufs=2, space="PSUM"))
            av_ps = tail_ctx.enter_context(
                tc.tile_pool(name="av_ps", bufs=1, space="PSUM"))
            for kt in range(8):
                tp = tp_ps.tile([128, 128], F16, tag="tp")
                nc.tensor.transpose(tp, E_sb[:, kt * 128:(kt + 1) * 128], ident)
                nc.vector.tensor_copy(ET_sb[:, kt, :], tp)
            ps_av = av_ps.tile([Q, D], F32)
            for kt in range(8):
                nc.tensor.matmul(ps_av, ET_sb[:, kt, :], val_sb[:, kt, :],
                                 start=(kt == 0), stop=(kt == 7))
            nc.vector.tensor_scalar_mul(out_sb, ps_av, rinv)
        nc.sync.dma_start(out=out_ext[:, :], in_=out_sb)

    _patch_multiwait(nc)
    return nc


def _patch_multiwait(nc):
    """walrus codegen on this toolchain accepts at most ONE sync wait per
    instruction ("Too many sync wait commands").  Tile emits up to 3 (and
    the kernel-tail Drain carries ~12).  Fix the serialized BIR:

    * DVE/Activation *compute* instructions waiting on their own engine's
      semaphore: the engine queue is in-order and drains between ops, so a
      same-engine wait is redundant - drop it.
    * Any instruction still holding >1 waits: hoist all but the last onto
      single-wait EventSemaphore carriers inserted just before it on the
      same engine queue (queue is in-order, so semantics are identical).
    """
    import json

    d = json.loads(nc.to_json_bytes())
    k = [0]
    self_drop = {"Activation": "Activation", "DVE": "DVE"}
    compute_ops = {"Activation", "TensorScalarPtr", "TensorScalar", "TensorTensor",
                   "TensorCopy", "TensorReduce", "Reciprocal", "Memset"}
    for fn in d["functions"]:
        for blk in fn["blocks"]:
            out = []
            for inst in blk["instructions"]:
                si = inst.get("sync_info") or {}
                ow = si.get("on_wait") or []
                op = inst.get("opcode")
                eng = inst.get("engine")
                if len(ow) > 1 and op != "EventSemaphore":
                    if op in compute_ops and eng in self_drop:
                        pref = self_drop[eng] + "_"
                        ow = [w for w in ow
                              if not str(w.get("ant_name", "")).startswith(pref)]
                    while len(ow) > 1:
                        w = ow.pop(0)
                        k[0] += 1
                        out.append({
                            "debug": inst.get("debug", 0), "engine": eng,
                            "ins": [], "name": f"WSplit-{k[0]}",
                            "opcode": "EventSemaphore", "outs": [],
                            "sync_info": {"on_update": [], "on_wait": [w]},
                        })
                    si["on_wait"] = ow
                out.append(inst)
            blk["instructions"] = out
    patched = json.dumps(d).encode()
    nc.to_json_bytes = lambda: patched


def _get_nc():
    if "nc" not in _CACHE:
        _CACHE["nc"] = _build_bass()
    return _CACHE["nc"]


def _host_prep(queries, keys, values, W_q, W_k, w_v, valid_lens):
    """Build the 8 per-core input maps."""
    queries = np.asarray(queries, dtype=np.float32)
    keys = np.asarray(keys, dtype=np.float32)
    values = np.asarray(values, dtype=np.float32)
    W_q = np.asarray(W_q, dtype=np.float32)
    W_k = np.asarray(W_k, dtype=np.float32)
    w_v = np.asarray(w_v, dtype=np.float32)
    valid = np.asarray(valid_lens).astype(np.int64)

    wqT = np.ascontiguousarray(W_q.T.astype(np.float16))     # (d, h)
    wkT = np.ascontiguousarray(W_k.T.astype(np.float16))
    wv_win = np.zeros((2, 128, 255), dtype=np.float16)
    wv_win[0, :, 127] = w_v[:128].astype(np.float16)
    wv_win[1, :, 127] = w_v[128:].astype(np.float16)

    kidx = np.arange(K)
    in_maps = []
    for b in range(B):
        mask = np.where(kidx < valid[b], np.float16(0.0), np.float16(NEG_BIG))
        in_maps.append({
            "qT": np.ascontiguousarray(queries[b].T.astype(np.float16)),
            "kT": np.ascontiguousarray(keys[b].T.astype(np.float16)),
            "vals": np.ascontiguousarray(values[b].astype(np.float16)),
            "wqT": wqT,
            "wkT": wkT,
            "wv_win": wv_win,
            "mask": np.ascontiguousarray(mask.reshape(1, K)),
        })
    return in_maps, valid, values


def _run(inputs, trace=False, **kw):
    from concourse.bass_utils import run_bass_kernel_spmd

    nc = _get_nc()
    in_maps, valid, values = _host_prep(**inputs)
    res = run_bass_kernel_spmd(nc, in_maps, list(range(B)), trace=trace, **kw)
    out = np.stack([np.asarray(res.results[i]["out"], dtype=np.float32)
                    for i in range(B)])
    # valid_len == 0 -> reference softmax over an all -1e9 row is uniform 1/K
    for b in range(B):
        if valid[b] == 0:
            out[b] = np.broadcast_to(values[b].mean(axis=0), (Q, D))
    return out, res


def kernel(**inputs):
    out, _ = _run(inputs, trace=False)
    return out


# revision 13
# speedup vs baseline: 1.1125x; 1.0197x over previous
"""Additive (Bahdanau) attention on 8 TRN2 NeuronCores.

Reference computation (per batch element b, one NeuronCore each):
    q  = queries @ W_q.T                      # (Q, H)
    k  = keys @ W_k.T                         # (K, H)
    s[q,k] = sum_h w_v[h] * tanh(q[q,h] + k[k,h])
    s[q,k] += mask (0 valid / -big invalid)
    attn = softmax_k(s)
    out  = attn @ values                      # (Q, Dv)

Shapes: B=8, Q=128, K=1024, D=256, H=256 (hardcoded; data-parallel over B).

Device strategy per core:
  * host pre-transposes/casts small operands to fp16 (qT, kT, W_q^T, W_k^T)
  * TensorE computes qf^T (h,q) and kf^T (h,k) projections
  * main loop over groups of G queries x 2 h-tiles:
      - VectorE  : arg[h, j*1024+k] = kfT[h,k] + qfT[h,q_j]  (tensor_scalar 4x)
      - ScalarE  : one (128, G*1024) tanh activation         (roofline engine)
      - TensorE  : h-reduction with w_v via a sliding-window one-hot-column
                   stationary, accumulating scores straight into (q,k) PSUM
  * mask folded in as a rank-1 matmul accumulate; softmax via reduce_max /
    activation(Exp, bias=-max, accum_out=rowsum); attn @ V via PE transposes.

ScalarE (tanh of Q*K*H = 33.5M elements) is the roofline engine: ~219us of
pure streaming at 1.2GHz/128 lanes; batching G queries per activation
amortizes the ~350-cycle per-instruction overhead.
"""

import numpy as np

B, Q, K, D, H = 8, 128, 1024, 256, 256
GQ = 8          # queries per activation instruction
NEG_BIG = -60000.0  # fp16-representable "minus infinity" for masking

_CACHE = {}


def _build_bass():
    import concourse.bass as bass
    import concourse.tile as tile
    from concourse import mybir
    from concourse.masks import make_identity
    from contextlib import ExitStack

    F32 = mybir.dt.float32
    F16 = mybir.dt.float16
    AF = mybir.ActivationFunctionType

    nc = bass.Bass()

    qT_ext = nc.declare_dram_parameter("qT", [D, Q], F16, isOutput=False)
    kT_ext = nc.declare_dram_parameter("kT", [D, K], F16, isOutput=False)
    vals_ext = nc.declare_dram_parameter("vals", [K, D], F16, isOutput=False)
    wqT_ext = nc.declare_dram_parameter("wqT", [D, H], F16, isOutput=False)
    wkT_ext = nc.declare_dram_parameter("wkT", [D, H], F16, isOutput=False)
    wv_ext = nc.declare_dram_parameter("wv_win", [2, 128, 255], F16, isOutput=False)
    mask_ext = nc.declare_dram_parameter("mask", [1, K], F16, isOutput=False)
    out_ext = nc.declare_dram_parameter("out", [Q, D], F32, isOutput=True)

    with tile.TileContext(nc) as tc, ExitStack() as ctx:
        persist = ctx.enter_context(tc.tile_pool(name="persist", bufs=1))
        scores_ps = ctx.enter_context(tc.tile_pool(name="scores_ps", bufs=1, space="PSUM"))
        arg_pool = ctx.enter_context(tc.tile_pool(name="arg_pool", bufs=3))
        t_pool = ctx.enter_context(tc.tile_pool(name="t_pool", bufs=3))

        # ---- persistent SBUF tiles ----
        qT_sb = persist.tile([128, 2, Q], F16)      # [d_in_tile, d_tile, q]
        kT_sb = persist.tile([128, 2, K], F16)
        wqT_sb = persist.tile([128, 2, H], F16)
        wkT_sb = persist.tile([128, 2, H], F16)
        wv_sb = persist.tile([128, 2, 255], F16)    # sliding-window w_v columns
        val_sb = persist.tile([128, 8, D], F16)     # [k_in_tile, k_tile, v]
        mask_sb = persist.tile([1, K], F16)
        ones_sb = persist.tile([1, 128], F16)
        ident = persist.tile([128, 128], F16)
        qfT_sb = persist.tile([128, 2, Q], F32)     # [h_in_tile, h_tile, q]
        kf_sb0 = persist.tile([128, K], F16)        # kfT, h-tile 0
        kf_sb1 = persist.tile([128, K], F16)        # kfT, h-tile 1
        E_sb = persist.tile([128, K], F16)          # exp(scores - max), (q, k)
        ET_sb = persist.tile([128, 8, 128], F16)    # transposed E, [k_in_tile, k_tile, q]
        out_sb = persist.tile([Q, D], F32)
        rowmax = persist.tile([128, 1], F32)
        negmax = persist.tile([128, 1], F32)
        rowsum = persist.tile([128, 1], F32)
        rinv = persist.tile([128, 1], F32)

        # ---- DMA inputs (kf-projection chain first: it gates the first tanh) ----
        nc.sync.dma_start(out=kT_sb[:, 0, :], in_=kT_ext[0:128, :])
        nc.sync.dma_start(out=wkT_sb, in_=wkT_ext.rearrange("(t p) h -> p t h", p=128))
        nc.sync.dma_start(out=kT_sb[:, 1, :], in_=kT_ext[128:256, :])
        nc.sync.dma_start(out=qT_sb, in_=qT_ext.rearrange("(t p) q -> p t q", p=128))
        nc.sync.dma_start(out=wqT_sb, in_=wqT_ext.rearrange("(t p) h -> p t h", p=128))
        nc.sync.dma_start(out=wv_sb, in_=wv_ext.rearrange("t p c -> p t c"))
        nc.sync.dma_start(out=mask_sb, in_=mask_ext[:, :])
        nc.sync.dma_start(out=val_sb, in_=vals_ext.rearrange("(t p) v -> p t v", p=128))
        nc.vector.memset(ones_sb, 1.0)
        make_identity(nc, ident)

        # ---- scores PSUM (q, k) over 2 banks; mask as rank-1 accumulate ----
        scores = scores_ps.tile([128, K], F32)
        for c in range(2):
            csl = slice(c * 512, (c + 1) * 512)
            nc.tensor.matmul(scores[:, csl], ones_sb, mask_sb[:, csl],
                             start=True, stop=False)

        with ExitStack() as setup_ctx:
            setup_ps = setup_ctx.enter_context(
                tc.tile_pool(name="setup_ps", bufs=1, space="PSUM"))
            kf_ps = setup_ctx.enter_context(
                tc.tile_pool(name="kf_ps", bufs=1, space="PSUM"))

            # ---- projections: qfT[h, q] and kfT[h, k] -> SBUF ----
            ps_q = setup_ps.tile([128, 2 * Q], F32)
            for ht in range(2):
                hsl = slice(ht * 128, (ht + 1) * 128)
                qsl = slice(ht * Q, (ht + 1) * Q)
                nc.tensor.matmul(ps_q[:, qsl], wqT_sb[:, 0, hsl], qT_sb[:, 0, :],
                                 start=True, stop=False)
                nc.tensor.matmul(ps_q[:, qsl], wqT_sb[:, 1, hsl], qT_sb[:, 1, :],
                                 start=False, stop=True)
            nc.vector.tensor_copy(qfT_sb, ps_q.rearrange("p (t q) -> p t q", t=2))

            kf0 = kf_ps.tile([128, K], F32, tag="kf0")
            kf1 = kf_ps.tile([128, K], F32, tag="kf1")
            kf = [kf0, kf1]
            kf_sb = [kf_sb0, kf_sb1]
            for ht in range(2):
                hsl = slice(ht * 128, (ht + 1) * 128)
                for c in range(2):
                    csl = slice(c * 512, (c + 1) * 512)
                    nc.tensor.matmul(kf[ht][:, csl], wkT_sb[:, 0, hsl],
                                     kT_sb[:, 0, csl], start=True, stop=False)
                    nc.tensor.matmul(kf[ht][:, csl], wkT_sb[:, 1, hsl],
                                     kT_sb[:, 1, csl], start=False, stop=True)
                    nc.vector.tensor_copy(kf_sb[ht][:, csl], kf[ht][:, csl])

        # ---- main loop: add + tanh (batched over GQ queries) + h-reduction ----
        # Tapered group sizes: small groups at the head shorten the serial
        # lead-in (fewer adds before the first tanh); small groups at the
        # tail shorten the last-group matmul drain before the softmax.
        group_sizes = [4, 4] + [GQ] * ((Q - 16) // GQ) + [4, 4]
        assert sum(group_sizes) == Q
        q0 = 0
        n_groups = len(group_sizes)
        for g, gs in enumerate(group_sizes):
            for ht in range(2):
                arg = arg_pool.tile([128, gs * K], F16, tag="arg")
                for j in range(gs):
                    q = q0 + j
                    nc.vector.tensor_scalar_add(
                        arg[:, j * K:(j + 1) * K], kf_sb[ht],
                        qfT_sb[:, ht, q:q + 1])
                tt = t_pool.tile([128, gs * K], F16, tag="tt")
                nc.scalar.activation(tt, arg, AF.Tanh)
                for j in range(gs):
                    q = q0 + j
                    last = (g == n_groups - 1) and (ht == 1) and (j == gs - 1)
                    for c in range(2):
                        csl = slice(c * 512, (c + 1) * 512)
                        nc.tensor.matmul(
                            scores[:, csl],
                            wv_sb[:, ht, 127 - q:255 - q],
                            tt[:, j * K + c * 512: j * K + (c + 1) * 512],
                            start=False, stop=last)
            q0 += gs

        # ---- masked softmax ----
        nc.vector.tensor_reduce(rowmax, scores, axis=mybir.AxisListType.X,
                                op=mybir.AluOpType.max)
        nc.vector.tensor_scalar_mul(negmax, rowmax, -1.0)
        nc.scalar.activation(E_sb, scores, AF.Exp, bias=negmax, scale=1.0,
                             accum_out=rowsum)
        nc.vector.reciprocal(rinv, rowsum)

        # ---- attn @ values: transpose E, then accumulate over k tiles ----
        with ExitStack() as tail_ctx:
            tp_ps = tail_ctx.enter_context(
                tc.tile_pool(name="tp_ps", bufs=2, space="PSUM"))
            av_ps = tail_ctx.enter_context(
                tc.tile_pool(name="av_ps", bufs=1, space="PSUM"))
            for kt in range(8):
                tp = tp_ps.tile([128, 128], F16, tag="tp")
                nc.tensor.transpose(tp, E_sb[:, kt * 128:(kt + 1) * 128], ident)
                nc.vector.tensor_copy(ET_sb[:, kt, :], tp)
            ps_av = av_ps.tile([Q, D], F32)
            for kt in range(8):
                nc.tensor.matmul(ps_av, ET_sb[:, kt, :], val_sb[:, kt, :],
                                 start=(kt == 0), stop=(kt == 7))
            nc.vector.tensor_scalar_mul(out_sb, ps_av, rinv)
        nc.sync.dma_start(out=out_ext[:, :], in_=out_sb)

    _patch_multiwait(nc)
    return nc


def _patch_multiwait(nc):
    """walrus codegen on this toolchain accepts at most ONE sync wait per
    instruction ("Too many sync wait commands").  Tile emits up to 3 (and
    the kernel-tail Drain carries ~12).  Fix the serialized BIR:

    * DVE/Activation *compute* instructions waiting on their own engine's
      semaphore: the engine queue is in-order and drains between ops, so a
      same-engine wait is redundant - drop it.
    * Any instruction still holding >1 waits: hoist all but the last onto
      single-wait EventSemaphore carriers inserted just before it on the
      same engine queue (queue is in-order, so semantics are identical).
    """
    import json

    d = json.loads(nc.to_json_bytes())
    k = [0]
    self_drop = {"Activation": "Activation", "DVE": "DVE"}
    compute_ops = {"Activation", "TensorScalarPtr", "TensorScalar", "TensorTensor",
                   "TensorCopy", "TensorReduce", "Reciprocal", "Memset"}
    for fn in d["functions"]:
        for blk in fn["blocks"]:
            out = []
            for inst in blk["instructions"]:
                si = inst.get("sync_info") or {}
                ow = si.get("on_wait") or []
                op = inst.get("opcode")
                eng = inst.get("engine")
                if len(ow) > 1 and op != "EventSemaphore":
                    if op in compute_ops and eng in self_drop:
                        pref = self_drop[eng] + "_"
                        ow = [w for w in ow
                              if not str(w.get("ant_name", "")).startswith(pref)]
                    while len(ow) > 1:
                        w = ow.pop(0)
                        k[0] += 1
                        out.append({
                            "debug": inst.get("debug", 0), "engine": eng,
                            "ins": [], "name": f"WSplit-{k[0]}",
                            "opcode": "EventSemaphore", "outs": [],
                            "sync_info": {"on_update": [], "on_wait": [w]},
                        })
                    si["on_wait"] = ow
                out.append(inst)
            blk["instructions"] = out
    patched = json.dumps(d).encode()
    nc.to_json_bytes = lambda: patched


def _get_nc():
    if "nc" not in _CACHE:
        _CACHE["nc"] = _build_bass()
    return _CACHE["nc"]


def _host_prep(queries, keys, values, W_q, W_k, w_v, valid_lens):
    """Build the 8 per-core input maps."""
    queries = np.asarray(queries, dtype=np.float32)
    keys = np.asarray(keys, dtype=np.float32)
    values = np.asarray(values, dtype=np.float32)
    W_q = np.asarray(W_q, dtype=np.float32)
    W_k = np.asarray(W_k, dtype=np.float32)
    w_v = np.asarray(w_v, dtype=np.float32)
    valid = np.asarray(valid_lens).astype(np.int64)

    wqT = np.ascontiguousarray(W_q.T.astype(np.float16))     # (d, h)
    wkT = np.ascontiguousarray(W_k.T.astype(np.float16))
    wv_win = np.zeros((2, 128, 255), dtype=np.float16)
    wv_win[0, :, 127] = w_v[:128].astype(np.float16)
    wv_win[1, :, 127] = w_v[128:].astype(np.float16)

    kidx = np.arange(K)
    in_maps = []
    for b in range(B):
        mask = np.where(kidx < valid[b], np.float16(0.0), np.float16(NEG_BIG))
        in_maps.append({
            "qT": np.ascontiguousarray(queries[b].T.astype(np.float16)),
            "kT": np.ascontiguousarray(keys[b].T.astype(np.float16)),
            "vals": np.ascontiguousarray(values[b].astype(np.float16)),
            "wqT": wqT,
            "wkT": wkT,
            "wv_win": wv_win,
            "mask": np.ascontiguousarray(mask.reshape(1, K)),
        })
    return in_maps, valid, values


def _run(inputs, trace=False, **kw):
    from concourse.bass_utils import run_bass_kernel_spmd

    nc = _get_nc()
    in_maps, valid, values = _host_prep(**inputs)
    res = run_bass_kernel_spmd(nc, in_maps, list(range(B)), trace=trace, **kw)
    out = np.stack([np.asarray(res.results[i]["out"], dtype=np.float32)
                    for i in range(B)])
    # valid_len == 0 -> reference softmax over an all -1e9 row is uniform 1/K
    for b in range(B):
        if valid[b] == 0:
            out[b] = np.broadcast_to(values[b].mean(axis=0), (Q, D))
    return out, res


def kernel(**inputs):
    out, _ = _run(inputs, trace=False)
    return out
